# revision 1
# baseline (speedup 1.0000x reference)
"""Trainium2 Bass kernel: 2-layer single-head GAT (PyG GATConv semantics).

Distribution (8 NeuronCores, node-parallel, SPMD single program):
  - Host: add self-loops, sort nodes by in-degree, deal nodes round-robin
    over the 8 cores (so chunk k on every core holds nodes of ~equal degree
    -> identical compile-time slot counts across cores), build per-core
    padded-CSR gather offset tables (row id of each in-edge's source).
  - Device, per core:
      phase A: rows [h1 | as1 | ad1] = x @ [W1 | W1 a_s | W1 a_d] for own
               nodes (PE matmuls from host-staged xT), AllGather the
               36-float rows into a replicated table g1buf [NTOT+1, 36].
      phase B: per 128-node chunk: per-slot-column indirect-DMA gathers
               (HW contract: one gathered row per partition per
               instruction), attention w = exp(leaky_relu(as+ad)) with
               denominator via ACT Exp accum_out, weighted sum via DVE
               mul + strided reduce, ELU -> z; build layer-2 rows
               (z @ [W2|...]) via PE transpose + matmul; AllGather -> g2buf.
      phase C: same aggregation against g2buf -> xbar.
  - Host: concat per-core outputs, invert the node permutation.

The per-slot-column indirect DMA (~1.44 us/instruction, 128 rows each) is
the throughput limit; all compute hides behind it.
"""

import os
import sys
from contextlib import ExitStack

for _p in ("/opt/trn_rl_repo",):
    if os.path.isdir(_p) and _p not in sys.path:
        sys.path.insert(0, _p)

import numpy as np

import concourse.bass as bass
import concourse.tile as tile
from concourse import bacc, mybir
from concourse.bass_utils import run_bass_kernel_spmd
from concourse.masks import make_identity

F32 = mybir.dt.float32
I32 = mybir.dt.int32
AF = mybir.ActivationFunctionType
ALU = mybir.AluOpType

P = 128
C = 8
DIM = 32
ROWF = 36
COL_AS = 32
COL_AD = 33
NEG_SLOPE = 0.2
EPS = 1e-16
DUMMY_AS = -1.0e30
NCH_MAX = 1   # >1 miscomputes on HW (sim-only validated); gathers dominate anyway
SLOT_CAP = 140   # max gather slots per group

RUN_KWARGS: dict = {}


# --------------------------------------------------------------------------
def partition_graph(edge_index: np.ndarray, n_nodes: int) -> dict:
    src = np.asarray(edge_index[0], dtype=np.int64)
    dst = np.asarray(edge_index[1], dtype=np.int64)
    # self-loops are handled on-device from SBUF-resident own rows; the
    # slot table holds only the real edges.
    deg = np.bincount(dst, minlength=n_nodes) + 1  # +1 self-loop
    rank = np.argsort(deg, kind="stable")

    assert n_nodes % C == 0
    npc_raw = n_nodes // C
    ch = -(-npc_raw // P)
    npc = ch * P
    ntot = C * npc

    pos = np.empty(n_nodes, dtype=np.int64)
    pos[rank] = np.arange(n_nodes)
    core_of = pos % C
    lidx_of = pos // C
    pid_of = core_of * npc + lidx_of

    key = pid_of[dst]
    order = np.argsort(key, kind="stable")
    src_pid_sorted = pid_of[src[order]].astype(np.int32)
    key_s = key[order]

    counts = np.bincount(key_s, minlength=ntot)
    starts = np.zeros(ntot + 1, dtype=np.int64)
    np.cumsum(counts, out=starts[1:])
    slot = np.arange(key_s.size, dtype=np.int64) - starts[key_s]

    degp = counts.reshape(C, ch, P)
    dk = np.maximum(degp.max(axis=(0, 2)), 1).astype(np.int64)
    off0 = np.zeros(ch + 1, dtype=np.int64)
    np.cumsum(dk, out=off0[1:])
    s_total = int(off0[-1])

    offs = np.full((C, P, s_total), ntot, dtype=np.int32)
    ec = key_s // npc
    el = key_s % npc
    ek = el // P
    ep = el % P
    ecol = off0[ek] + slot
    offs[ec, ep, ecol] = src_pid_sorted

    # chunk groups: consecutive chunks, nch <= NCH_MAX, slot-sum <= SLOT_CAP
    groups = []
    c0 = 0
    while c0 < ch:
        c1 = c0 + 1
        while (c1 < ch and c1 - c0 < NCH_MAX
               and off0[c1 + 1] - off0[c0] <= SLOT_CAP):
            c1 += 1
        groups.append((c0, c1))
        c0 = c1

    return dict(
        rank=rank, npc_raw=npc_raw, NPC=npc, CH=ch, NTOT=ntot,
        Dk=dk, off0=off0, S=s_total, offs=offs, groups=groups,
    )


# --------------------------------------------------------------------------
def build_program(meta: dict):
    npc, ch, ntot = meta["NPC"], meta["CH"], meta["NTOT"]
    dk, off0 = meta["Dk"], meta["off0"]
    s_total, groups = meta["S"], meta["groups"]
    r_rows = ntot + 1

    nc = bacc.Bacc("TRN2", target_bir_lowering=False, debug=False, num_devices=C)

    xT_d = nc.dram_tensor("xT", [DIM, npc], F32, kind="ExternalInput").ap()
    offs_d = nc.dram_tensor("offs", [P, s_total], I32, kind="ExternalInput").ap()
    m1_d = nc.dram_tensor("m1aug", [DIM, ROWF], F32, kind="ExternalInput").ap()
    m2_d = nc.dram_tensor("m2aug", [2 * DIM, ROWF], F32, kind="ExternalInput").ap()
    b1_d = nc.dram_tensor("b1rep", [P, DIM], F32, kind="ExternalInput").ap()
    b2_d = nc.dram_tensor("b2rep", [P, DIM], F32, kind="ExternalInput").ap()
    dum_d = nc.dram_tensor("dummyrow", [1, ROWF], F32, kind="ExternalInput").ap()
    out_z = nc.dram_tensor("out_z", [npc, DIM], F32, kind="ExternalOutput").ap()
    out_x = nc.dram_tensor("out_xbar", [npc, DIM], F32, kind="ExternalOutput").ap()

    cc1 = nc.dram_tensor("cc1", [npc, ROWF], F32).ap()
    cc2 = nc.dram_tensor("cc2", [npc, ROWF], F32).ap()
    g1 = nc.dram_tensor("g1buf", [r_rows, ROWF], F32, addr_space="Shared").ap()
    g2 = nc.dram_tensor("g2buf", [r_rows, ROWF], F32, addr_space="Shared").ap()
    rg = [list(range(C))]

    with tile.TileContext(nc) as tc:
        with ExitStack() as ctx:
            const = ctx.enter_context(tc.tile_pool(name="const", bufs=1))
            stg = ctx.enter_context(tc.tile_pool(name="stg", bufs=3))
            gpool = ctx.enter_context(tc.tile_pool(name="gpool", bufs=2))
            wpool = ctx.enter_context(tc.tile_pool(name="wpool", bufs=3))
            ppool = ctx.enter_context(tc.tile_pool(name="ppool", bufs=3))
            npool = ctx.enter_context(tc.tile_pool(name="npool", bufs=4))
            psA = ctx.enter_context(tc.tile_pool(name="psA", bufs=3, space="PSUM"))
            psT = ctx.enter_context(tc.tile_pool(name="psT", bufs=2, space="PSUM"))

            xT_s = const.tile([DIM, npc], F32)
            nc.sync.dma_start(out=xT_s[:], in_=xT_d)
            offs_s = const.tile([P, s_total], I32)
            nc.sync.dma_start(out=offs_s[:], in_=offs_d)
            m1_s = const.tile([DIM, ROWF], F32)
            nc.sync.dma_start(out=m1_s[:], in_=m1_d)
            m2_s = const.tile([2 * DIM, ROWF], F32)
            nc.sync.dma_start(out=m2_s[:], in_=m2_d)
            b1_s = const.tile([P, DIM], F32)
            nc.sync.dma_start(out=b1_s[:], in_=b1_d)
            b2_s = const.tile([P, DIM], F32)
            nc.sync.dma_start(out=b2_s[:], in_=b2_d)
            dum_s = const.tile([1, ROWF], F32)
            nc.sync.dma_start(out=dum_s[:], in_=dum_d)
            ident = const.tile([P, P], F32)
            make_identity(nc, ident[:])
            ad1_s = const.tile([P, ch], F32)
            ad2_s = const.tile([P, ch], F32)
            own1_s = const.tile([P, ch * ROWF], F32)
            own2_s = const.tile([P, ch * ROWF], F32)

            nc.sync.dma_start(out=g1[ntot:ntot + 1, :], in_=dum_s[:])
            nc.sync.dma_start(out=g2[ntot:ntot + 1, :], in_=dum_s[:])

            def dram_rows(dram, c0, nch, rowf):
                """[128, nch, rowf] view of dram rows c0*128 .. (c0+nch)*128."""
                return dram[c0 * P:(c0 + nch) * P, :].rearrange(
                    "(c p) f -> p c f", p=P)

            # ---- phase A: layer-1 rows for own nodes ----
            for (c0, c1) in groups:
                nch = c1 - c0
                ps = psA.tile([P, nch * ROWF], F32, tag="psA")
                for j in range(nch):
                    nc.tensor.matmul(
                        out=ps[:, j * ROWF:(j + 1) * ROWF],
                        lhsT=xT_s[:, (c0 + j) * P:(c0 + j + 1) * P],
                        rhs=m1_s[:], start=True, stop=True)
                gs = own1_s[:, c0 * ROWF:c1 * ROWF]
                nc.vector.tensor_copy(out=gs, in_=ps[:])
                ps3 = ps[:].rearrange("p (c f) -> p c f", f=ROWF)
                nc.vector.tensor_copy(out=ad1_s[:, c0:c1], in_=ps3[:, :, COL_AD])
                nc.sync.dma_start(
                    out=dram_rows(cc1, c0, nch, ROWF),
                    in_=gs.rearrange("p (c f) -> p c f", f=ROWF))

            nc.gpsimd.collective_compute(
                "AllGather", ALU.bypass, replica_groups=rg,
                ins=[cc1], outs=[g1[0:ntot, :]])

            # ---- grouped aggregation ----
            def aggregate_group(c0, c1, gbuf, ad_s, b_s, own_s):
                nch = c1 - c0
                a0 = int(off0[c0])
                sg = int(off0[c1]) - a0  # total slots in group
                gt = gpool.tile([P, sg * ROWF], F32, tag="gt")
                # HW indirect DMA gathers one row per partition per
                # instruction: one gather per slot column.
                for q in range(sg):
                    nc.gpsimd.indirect_dma_start(
                        out=gt[:, q * ROWF:(q + 1) * ROWF],
                        out_offset=None, in_=gbuf,
                        in_offset=bass.IndirectOffsetOnAxis(
                            ap=offs_s[:, a0 + q:a0 + q + 1], axis=0))
                g3 = gt[:].rearrange("p (s f) -> p s f", f=ROWF)
                w_g = wpool.tile([P, sg], F32, tag="w")
                den = npool.tile([P, nch], F32, tag="den")
                rden = npool.tile([P, nch], F32, tag="rden")
                opre = npool.tile([P, nch * DIM], F32, tag="opre")
                for j in range(nch):
                    c = c0 + j
                    d = int(dk[c])
                    a = int(off0[c]) - a0
                    s = wpool.tile([P, d], F32, tag="s")
                    nc.vector.tensor_tensor(
                        out=s[:], in0=g3[:, a:a + d, COL_AS],
                        in1=ad_s[:, c:c + 1].to_broadcast([P, d]), op=ALU.add)
                    nc.vector.scalar_tensor_tensor(
                        out=s[:], in0=s[:], scalar=NEG_SLOPE, in1=s[:],
                        op0=ALU.mult, op1=ALU.max)
                    nc.scalar.activation(
                        out=w_g[:, a:a + d], in_=s[:], func=AF.Exp,
                        accum_out=den[:, j:j + 1])
                # self-loop term from the SBUF-resident own rows
                ws = npool.tile([P, nch], F32, tag="ws")
                own3 = own_s[:, c0 * ROWF:c1 * ROWF].rearrange(
                    "p (c f) -> p c f", f=ROWF)
                nc.vector.tensor_tensor(
                    out=ws[:], in0=own3[:, :, COL_AS], in1=ad_s[:, c0:c1],
                    op=ALU.add)
                nc.vector.scalar_tensor_tensor(
                    out=ws[:], in0=ws[:], scalar=NEG_SLOPE, in1=ws[:],
                    op0=ALU.mult, op1=ALU.max)
                nc.scalar.activation(out=ws[:], in_=ws[:], func=AF.Exp)
                nc.vector.tensor_add(out=den[:], in0=den[:], in1=ws[:])
                nc.vector.tensor_scalar_add(out=den[:], in0=den[:], scalar1=EPS)
                nc.vector.reciprocal(out=rden[:], in_=den[:])
                for j in range(nch):
                    c = c0 + j
                    d = int(dk[c])
                    a = int(off0[c]) - a0
                    prod = ppool.tile([P, d * DIM], F32, tag="prod")
                    nc.vector.tensor_tensor(
                        out=prod[:].rearrange("p (d c1) -> p d c1", c1=DIM),
                        in0=g3[:, a:a + d, 0:DIM],
                        in1=w_g[:, a:a + d].unsqueeze(2).to_broadcast([P, d, DIM]),
                        op=ALU.mult)
                    num = npool.tile([P, DIM], F32, tag="num")
                    nc.vector.tensor_reduce(
                        out=num[:],
                        in_=prod[:].rearrange("p (d c1) -> p c1 d", c1=DIM),
                        axis=mybir.AxisListType.X, op=ALU.add)
                    nc.vector.scalar_tensor_tensor(
                        out=num[:], in0=own3[:, j, 0:DIM],
                        scalar=ws[:, j:j + 1], in1=num[:],
                        op0=ALU.mult, op1=ALU.add)
                    nc.vector.scalar_tensor_tensor(
                        out=opre[:, j * DIM:(j + 1) * DIM], in0=num[:],
                        scalar=rden[:, j:j + 1], in1=b_s[:],
                        op0=ALU.mult, op1=ALU.add)
                # elu(x) = max(x, exp(min(x, 0)) - 1), group-batched
                mneg = npool.tile([P, nch * DIM], F32, tag="mneg")
                nc.vector.tensor_scalar_min(out=mneg[:], in0=opre[:], scalar1=0.0)
                e = npool.tile([P, nch * DIM], F32, tag="e")
                nc.scalar.activation(out=e[:], in_=mneg[:], func=AF.Exp)
                z_g = npool.tile([P, nch * DIM], F32, tag="z")
                nc.vector.scalar_tensor_tensor(
                    out=z_g[:], in0=e[:], scalar=-1.0, in1=opre[:],
                    op0=ALU.add, op1=ALU.max)
                return z_g

            # ---- phase B ----
            for (c0, c1) in groups:
                nch = c1 - c0
                z_g = aggregate_group(c0, c1, g1, ad1_s, b1_s, own1_s)
                nc.sync.dma_start(
                    out=dram_rows(out_z, c0, nch, DIM),
                    in_=z_g[:].rearrange("p (c f) -> p c f", f=DIM))
                ps2 = psA.tile([P, nch * ROWF], F32, tag="psA")
                for h in range(0, nch, 2):
                    nh = min(2, nch - h)
                    ptT = psT.tile([nh * DIM, P], F32, tag="psT")
                    nc.tensor.transpose(
                        out=ptT[:], in_=z_g[:, h * DIM:(h + nh) * DIM],
                        identity=ident[:])
                    zT = stg.tile([nh * DIM, P], F32, tag="zT")
                    nc.vector.tensor_copy(out=zT[:], in_=ptT[:])
                    for j in range(nh):
                        nc.tensor.matmul(
                            out=ps2[:, (h + j) * ROWF:(h + j + 1) * ROWF],
                            lhsT=zT[j * DIM:(j + 1) * DIM, :],
                            rhs=m2_s[j * DIM:(j + 1) * DIM, :],
                            start=True, stop=True)
                gs2 = own2_s[:, c0 * ROWF:c1 * ROWF]
                nc.vector.tensor_copy(out=gs2, in_=ps2[:])
                ps23 = ps2[:].rearrange("p (c f) -> p c f", f=ROWF)
                nc.vector.tensor_copy(out=ad2_s[:, c0:c1], in_=ps23[:, :, COL_AD])
                nc.sync.dma_start(
                    out=dram_rows(cc2, c0, nch, ROWF),
                    in_=gs2.rearrange("p (c f) -> p c f", f=ROWF))

            nc.gpsimd.collective_compute(
                "AllGather", ALU.bypass, replica_groups=rg,
                ins=[cc2], outs=[g2[0:ntot, :]])

            # ---- phase C ----
            for (c0, c1) in groups:
                nch = c1 - c0
                x_g = aggregate_group(c0, c1, g2, ad2_s, b2_s, own2_s)
                nc.sync.dma_start(
                    out=dram_rows(out_x, c0, nch, DIM),
                    in_=x_g[:].rearrange("p (c f) -> p c f", f=DIM))

    nc.compile()
    return nc


# --------------------------------------------------------------------------
def make_in_maps(meta, x, W1, a_src1, a_dst1, b1, W2, a_src2, a_dst2, b2):
    rank, npc_raw, npc = meta["rank"], meta["npc_raw"], meta["NPC"]

    def maug(w, a_s, a_d):
        m = np.zeros((DIM, ROWF), dtype=np.float32)
        m[:, 0:DIM] = w
        m[:, COL_AS] = w @ a_s
        m[:, COL_AD] = w @ a_d
        return m

    m1 = maug(W1, a_src1, a_dst1)
    m2 = np.tile(maug(W2, a_src2, a_dst2), (2, 1))
    b1r = np.tile(b1.astype(np.float32)[None, :], (P, 1))
    b2r = np.tile(b2.astype(np.float32)[None, :], (P, 1))
    dummy = np.zeros((1, ROWF), dtype=np.float32)
    dummy[0, COL_AS] = DUMMY_AS

    xm = x[rank]
    in_maps = []
    for c in range(C):
        xc = xm[c::C]
        xT = np.zeros((DIM, npc), dtype=np.float32)
        xT[:, :npc_raw] = xc.T
        in_maps.append({
            "xT": np.ascontiguousarray(xT),
            "offs": np.ascontiguousarray(meta["offs"][c]),
            "m1aug": m1, "m2aug": m2, "b1rep": b1r, "b2rep": b2r,
            "dummyrow": dummy,
        })
    return in_maps


def unshard(meta, results, n_nodes):
    rank, npc_raw = meta["rank"], meta["npc_raw"]
    z = np.empty((n_nodes, DIM), dtype=np.float32)
    xbar = np.empty((n_nodes, DIM), dtype=np.float32)
    for c in range(C):
        ids = rank[np.arange(npc_raw) * C + c]
        z[ids] = results[c]["out_z"][:npc_raw]
        xbar[ids] = results[c]["out_xbar"][:npc_raw]
    return xbar, z


_CACHE: dict = {}


def kernel(x, edge_index, W1, a_src1, a_dst1, b1, W2, a_src2, a_dst2, b2):
    x = np.asarray(x, dtype=np.float32)
    edge_index = np.asarray(edge_index)
    args = [np.asarray(a, dtype=np.float32)
            for a in (W1, a_src1, a_dst1, b1, W2, a_src2, a_dst2, b2)]
    n_nodes = x.shape[0]

    import time as _time
    t0 = _time.time()
    meta = partition_graph(edge_index, n_nodes)
    in_maps = make_in_maps(meta, x, *args)
    t1 = _time.time()

    key = (n_nodes, meta["S"], tuple(meta["Dk"].tolist()))
    nc = _CACHE.get(key)
    if nc is None:
        nc = build_program(meta)
        _CACHE[key] = nc
    t2 = _time.time()
    print(f"[kernel] partition {t1 - t0:.1f}s  build+compile {t2 - t1:.1f}s",
          flush=True)

    res = run_bass_kernel_spmd(nc, in_maps, list(range(C)), **RUN_KWARGS)
    xbar, z = unshard(meta, res.results, n_nodes)
    if RUN_KWARGS:
        kernel.last_result = res  # type: ignore[attr-defined]
    return xbar, z



# revision 12
# speedup vs baseline: 1.6894x; 1.6894x over previous
"""Trainium2 Bass kernel: 2-layer single-head GAT (PyG GATConv semantics).

Distribution (8 NeuronCores, node-parallel, SPMD single program):
  - Host: add self-loops, sort nodes by in-degree, deal nodes round-robin
    over the 8 cores, build per-core padded-CSR gather tables (slot-column
    layout, one gathered source row per dst lane per slot column).
  - Device, per core and per layer:
      rows [h | as | ad] for own nodes via PE matmuls; h cast to bf16 and
      AllGathered into a replicated node table g [NTOT, 32] bf16.
      Aggregation per chunk group: slot sources fetched with 4-queue
      dma_gather (InstDMAGatherAnt) at 256B granularity -- one gathered
      element = 4 consecutive bf16 node rows (a "quad", idx = pid//4 fits
      int16).  A 4-pass masked select on DVE extracts each slot's node from
      its quad; as = h . a_src is recomputed on the fly.  Attention
      w = exp(leaky_relu(as + ad)), weighted sum via DVE bf16 mul + strided
      reduce, self-loop terms applied from SBUF-resident f32 own rows.
  - Host: concat per-core outputs, invert the node permutation.

dma_gather replaces the per-slot-column indirect DMA of the previous
version: ~2.5-3 ns per gathered row vs ~11 ns (the 994 ns SWDGE fixed cost
amortizes over 1024-index calls spread across 4 SWDGE queues).
"""

import os
import sys
from contextlib import ExitStack

for _p in ("/opt/trn_rl_repo",):
    if os.path.isdir(_p) and _p not in sys.path:
        sys.path.insert(0, _p)

import ml_dtypes
import numpy as np

import concourse.bass as bass
import concourse.tile as tile
from concourse import bacc, library_config, mybir
from concourse.bass_utils import run_bass_kernel_spmd
from concourse.masks import make_identity

F32 = mybir.dt.float32
BF16 = mybir.dt.bfloat16
I16 = mybir.dt.int16
AF = mybir.ActivationFunctionType
ALU = mybir.AluOpType

P = 128
C = 8
DIM = 32
ROWF = 36
COL_AS = 32
COL_AD = 33
NEG_SLOPE = 0.2
EPS = 1e-16
NCH_MAX = 3      # chunks per group
COL_CAP = 64     # slot columns per group (gather tile = COL_CAP*256B/part)
CALL_COLS = 8    # slot columns per dma_gather call (8*128 = 1024 idxs)
NQ = 4           # SWDGE queues

RUN_KWARGS: dict = {}


# --------------------------------------------------------------------------
def partition_graph(edge_index: np.ndarray, n_nodes: int) -> dict:
    src = np.asarray(edge_index[0], dtype=np.int64)
    dst = np.asarray(edge_index[1], dtype=np.int64)
    # self-loops handled on-device from SBUF-resident own rows.
    deg = np.bincount(dst, minlength=n_nodes) + 1
    rank = np.argsort(deg, kind="stable")

    assert n_nodes % C == 0
    npc_raw = n_nodes // C
    ch = -(-npc_raw // P)
    npc = ch * P
    assert npc % 4 == 0
    ntot = C * npc

    pos = np.empty(n_nodes, dtype=np.int64)
    pos[rank] = np.arange(n_nodes)
    core_of = pos % C
    lidx_of = pos // C
    pid_of = core_of * npc + lidx_of

    key = pid_of[dst]
    order = np.argsort(key, kind="stable")
    src_pid_sorted = pid_of[src[order]].astype(np.int32)
    key_s = key[order]

    counts = np.bincount(key_s, minlength=ntot)
    starts = np.zeros(ntot + 1, dtype=np.int64)
    np.cumsum(counts, out=starts[1:])
    slot = np.arange(key_s.size, dtype=np.int64) - starts[key_s]

    degp = counts.reshape(C, ch, P)
    dk = np.maximum(degp.max(axis=(0, 2)), 1).astype(np.int64)
    off0 = np.zeros(ch + 1, dtype=np.int64)
    np.cumsum(dk, out=off0[1:])
    s_total = int(off0[-1])

    offs = np.full((C, P, s_total), ntot, dtype=np.int64)
    ec = key_s // npc
    el = key_s % npc
    ek = el // P
    ep = el % P
    ecol = off0[ek] + slot
    offs[ec, ep, ecol] = src_pid_sorted

    # chunk groups + per-group dma_gather call plan
    nch_max = int(os.environ.get("GAT_NCH", str(NCH_MAX)))
    groups = []
    c0 = 0
    while c0 < ch:
        c1 = c0 + 1
        while (c1 < ch and c1 - c0 < nch_max
               and off0[c1 + 1] - off0[c0] <= COL_CAP):
            c1 += 1
        groups.append((c0, c1))
        c0 = c1

    calls = []  # (group_idx, idx_off_int16cols, col_off_in_group, ncols)
    idx_off = 0
    for gi, (c0, c1) in enumerate(groups):
        gcols = int(off0[c1] - off0[c0])
        coff = 0
        while coff < gcols:
            ncols = min(CALL_COLS, gcols - coff)
            calls.append((gi, idx_off, coff, ncols))
            idx_off += ncols * 8  # ncols*128/16 int16 per partition
            coff += ncols
    idxw = idx_off

    # per-core int16 wrapped idx arrays + masks
    qtot = ntot // 4
    qidx = np.empty((C, P, idxw), dtype=np.int16)
    for c in range(C):
        quads = (offs[c] // 4).astype(np.int16)  # pad -> ntot//4 = dummy quad
        for (gi, ioff, coff, ncols) in calls:
            a0 = int(off0[groups[gi][0]]) + coff
            flat = quads[:, a0:a0 + ncols].T.reshape(-1)  # [ncols*128] j*128+p
            wrapped = flat.reshape(-1, 16).T              # [16, ncols*8]
            qidx[c, :, ioff:ioff + ncols * 8] = np.tile(wrapped, (8, 1))

    quarter = (offs % 4).astype(np.int64)
    validm = offs != ntot
    masks = np.zeros((C, P, 4 * s_total), dtype=ml_dtypes.bfloat16)
    for s in range(4):
        masks[:, :, s * s_total:(s + 1) * s_total] = (
            validm & (quarter == s)).astype(ml_dtypes.bfloat16)
    valid = validm.astype(np.float32)

    return dict(
        rank=rank, npc_raw=npc_raw, NPC=npc, CH=ch, NTOT=ntot, QTOT=qtot,
        Dk=dk, off0=off0, S=s_total, groups=groups, calls=calls, IDXW=idxw,
        qidx=qidx, masks=masks, valid=valid,
    )


# --------------------------------------------------------------------------
def build_program(meta: dict):
    npc, ch, ntot = meta["NPC"], meta["CH"], meta["NTOT"]
    dk, off0 = meta["Dk"], meta["off0"]
    s_total, groups, calls = meta["S"], meta["groups"], meta["calls"]
    idxw = meta["IDXW"]
    qtot = meta["QTOT"]

    nq = int(os.environ.get("GAT_NQ", str(NQ)))
    scratch = int(os.environ.get("GAT_SCRATCH", "32768"))
    nc = bacc.Bacc("TRN2", target_bir_lowering=False, debug=False,
                   num_devices=C, num_swdge_queues=nq,
                   dynamic_dma_scratch_size=scratch)

    xT_d = nc.dram_tensor("xT", [DIM, npc], F32, kind="ExternalInput").ap()
    qidx_d = nc.dram_tensor("qidx", [P, idxw], I16, kind="ExternalInput").ap()
    mask_d = nc.dram_tensor("masks", [P, 4 * s_total], BF16,
                            kind="ExternalInput").ap()
    valid_d = nc.dram_tensor("valid", [P, s_total], F32,
                             kind="ExternalInput").ap()
    m1_d = nc.dram_tensor("m1aug", [DIM, ROWF], F32, kind="ExternalInput").ap()
    m2_d = nc.dram_tensor("m2aug", [2 * DIM, ROWF], F32,
                          kind="ExternalInput").ap()
    asr1_d = nc.dram_tensor("asr1", [P, DIM], BF16, kind="ExternalInput").ap()
    asr2_d = nc.dram_tensor("asr2", [P, DIM], BF16, kind="ExternalInput").ap()
    b1_d = nc.dram_tensor("b1rep", [P, DIM], F32, kind="ExternalInput").ap()
    b2_d = nc.dram_tensor("b2rep", [P, DIM], F32, kind="ExternalInput").ap()
    out_z = nc.dram_tensor("out_z", [npc, DIM], F32, kind="ExternalOutput").ap()
    out_x = nc.dram_tensor("out_xbar", [npc, DIM], F32,
                           kind="ExternalOutput").ap()

    cc1 = nc.dram_tensor("cc1", [npc, DIM], BF16).ap()
    cc2 = nc.dram_tensor("cc2", [npc, DIM], BF16).ap()
    g1 = nc.dram_tensor("g1buf", [ntot + 4, DIM], BF16, addr_space="Shared").ap()
    g2 = nc.dram_tensor("g2buf", [ntot + 4, DIM], BF16, addr_space="Shared").ap()
    rg = [list(range(C))]

    private_table = os.environ.get("GAT_PRIVATE_TABLE", "1") == "1"
    if private_table:
        g1p = nc.dram_tensor("g1p", [ntot + 4, DIM], BF16).ap()
        g2p = nc.dram_tensor("g2p", [ntot + 4, DIM], BF16).ap()
    else:
        g1p, g2p = g1, g2

    # quad views: one 256B row = 4 consecutive bf16 node rows
    g1q = g1p.rearrange("(q s) f -> q (s f)", s=4)
    g2q = g2p.rearrange("(q s) f -> q (s f)", s=4)

    with tile.TileContext(nc) as tc:
        with ExitStack() as ctx:
            const = ctx.enter_context(tc.tile_pool(name="const", bufs=1))
            stg = ctx.enter_context(tc.tile_pool(name="stg", bufs=3))
            gpool = ctx.enter_context(tc.tile_pool(name="gpool", bufs=2))
            hpool = ctx.enter_context(tc.tile_pool(name="hpool", bufs=3))
            wpool = ctx.enter_context(tc.tile_pool(name="wpool", bufs=4))
            npool = ctx.enter_context(tc.tile_pool(name="npool", bufs=4))
            psA = ctx.enter_context(tc.tile_pool(name="psA", bufs=3, space="PSUM"))
            psT = ctx.enter_context(tc.tile_pool(name="psT", bufs=2, space="PSUM"))

            if os.environ.get("GAT_NO_LOADLIB", "0") != "1":
                nc.gpsimd.load_library(library_config.mlp)

            xT_s = const.tile([DIM, npc], F32)
            nc.sync.dma_start(out=xT_s[:], in_=xT_d)
            qidx_s = const.tile([P, idxw], I16)
            nc.sync.dma_start(out=qidx_s[:], in_=qidx_d)
            mask_s = const.tile([P, 4 * s_total], BF16)
            nc.sync.dma_start(out=mask_s[:], in_=mask_d)
            valid_s = const.tile([P, s_total], F32)
            nc.sync.dma_start(out=valid_s[:], in_=valid_d)
            m1_s = const.tile([DIM, ROWF], F32)
            nc.sync.dma_start(out=m1_s[:], in_=m1_d)
            m2_s = const.tile([2 * DIM, ROWF], F32)
            nc.sync.dma_start(out=m2_s[:], in_=m2_d)
            asr1_s = const.tile([P, DIM], BF16)
            nc.sync.dma_start(out=asr1_s[:], in_=asr1_d)
            asr2_s = const.tile([P, DIM], BF16)
            nc.sync.dma_start(out=asr2_s[:], in_=asr2_d)
            b1_s = const.tile([P, DIM], F32)
            nc.sync.dma_start(out=b1_s[:], in_=b1_d)
            b2_s = const.tile([P, DIM], F32)
            nc.sync.dma_start(out=b2_s[:], in_=b2_d)
            ident = const.tile([P, P], F32)
            make_identity(nc, ident[:])
            zer = const.tile([1, 4 * DIM], BF16)
            nc.vector.memset(zer[:], 0.0)
            ad1_s = const.tile([P, ch], F32)
            ad2_s = const.tile([P, ch], F32)
            own1_s = const.tile([P, ch * ROWF], F32)
            own2_s = const.tile([P, ch * ROWF], F32)

            # dummy quad row (zeros) at quad index qtot
            nc.sync.dma_start(out=g1q[qtot:qtot + 1, :], in_=zer[:])
            nc.sync.dma_start(out=g2q[qtot:qtot + 1, :], in_=zer[:])

            def mirror(shared, priv):
                if private_table:
                    nc.sync.dma_start(
                        out=priv[0:ntot, :].rearrange("(o p) f -> p o f", p=P),
                        in_=shared[0:ntot, :].rearrange("(o p) f -> p o f", p=P))

            def dram_rows(dram, c0, nch, rowf):
                return dram[c0 * P:(c0 + nch) * P, :].rearrange(
                    "(c p) f -> p c f", p=P)

            # ---- phase A: layer-1 rows for own nodes ----
            for (c0, c1) in groups:
                nch = c1 - c0
                ps = psA.tile([P, nch * ROWF], F32, tag="psA")
                for j in range(nch):
                    nc.tensor.matmul(
                        out=ps[:, j * ROWF:(j + 1) * ROWF],
                        lhsT=xT_s[:, (c0 + j) * P:(c0 + j + 1) * P],
                        rhs=m1_s[:], start=True, stop=True)
                gs = own1_s[:, c0 * ROWF:c1 * ROWF]
                nc.vector.tensor_copy(out=gs, in_=ps[:])
                ps3 = ps[:].rearrange("p (c f) -> p c f", f=ROWF)
                nc.vector.tensor_copy(out=ad1_s[:, c0:c1], in_=ps3[:, :, COL_AD])
                hb = stg.tile([P, nch * DIM], BF16, tag="hb")
                nc.vector.tensor_copy(
                    out=hb[:].rearrange("p (c f) -> p c f", f=DIM),
                    in_=ps3[:, :, 0:DIM])
                nc.sync.dma_start(
                    out=dram_rows(cc1, c0, nch, DIM),
                    in_=hb[:].rearrange("p (c f) -> p c f", f=DIM))

            nc.gpsimd.collective_compute(
                "AllGather", ALU.bypass, replica_groups=rg,
                ins=[cc1], outs=[g1[0:ntot, :]])
            mirror(g1, g1p)

            # ---- grouped aggregation ----
            qcall = [0]

            def aggregate_group(gi, gbuf_q, asr_s, ad_s, b_s, own_s):
                c0, c1 = groups[gi]
                nch = c1 - c0
                a0 = int(off0[c0])
                gcols = int(off0[c1]) - a0
                gt = gpool.tile([P, gcols * 4 * DIM], BF16, tag="gt")
                gt3 = gt[:].rearrange("p (s f) -> p s f", f=4 * DIM)
                if os.environ.get("GAT_NO_GATHER", "0") == "1":
                    nc.vector.memset(gt[:], 0.0)
                else:
                    for (gj, ioff, coff, ncols) in calls:
                        if gj != gi:
                            continue
                        ni = ncols * P
                        nc.gpsimd.dma_gather(
                            gt3[:, coff:coff + ncols, :], gbuf_q,
                            qidx_s[:, ioff:ioff + ncols * 8], ni, ni, 4 * DIM,
                            queue_num=qcall[0] % NQ)
                        qcall[0] += 1
                g4 = gt[:].rearrange("p (q s f) -> p q s f", s=4, f=DIM)

                own3 = own_s[:, c0 * ROWF:c1 * ROWF].rearrange(
                    "p (c f) -> p c f", f=ROWF)
                ws = npool.tile([P, nch], F32, tag="ws")
                nc.vector.tensor_tensor(
                    out=ws[:], in0=own3[:, :, COL_AS], in1=ad_s[:, c0:c1],
                    op=ALU.add)
                nc.vector.scalar_tensor_tensor(
                    out=ws[:], in0=ws[:], scalar=NEG_SLOPE, in1=ws[:],
                    op0=ALU.mult, op1=ALU.max)
                nc.scalar.activation(out=ws[:], in_=ws[:], func=AF.Exp)

                den = npool.tile([P, nch], F32, tag="den")
                rden = npool.tile([P, nch], F32, tag="rden")
                opre = npool.tile([P, nch * DIM], F32, tag="opre")
                for j in range(nch):
                    c = c0 + j
                    d = int(dk[c])
                    a = int(off0[c]) - a0   # cols into group tile
                    A = int(off0[c])        # global slot col
                    # hsel = sum_s g4[:, :, s, :] * mask_s   (bf16)
                    hsel = hpool.tile([P, d * DIM], BF16, tag="hsel")
                    h3 = hsel[:].rearrange("p (q f) -> p q f", f=DIM)
                    tb = hpool.tile([P, d * DIM], BF16, tag="tb")
                    t3 = tb[:].rearrange("p (q f) -> p q f", f=DIM)
                    for s in range(4):
                        m = mask_s[:, s * s_total + A:s * s_total + A + d]
                        dst3 = h3 if s == 0 else t3
                        nc.vector.tensor_tensor(
                            out=dst3, in0=g4[:, a:a + d, s, :],
                            in1=m.unsqueeze(2).to_broadcast([P, d, DIM]),
                            op=ALU.mult)
                        if s:
                            nc.vector.tensor_add(out=hsel[:], in0=hsel[:],
                                                 in1=tb[:])
                    # as = reduce_f(hsel * a_src)
                    nc.vector.tensor_tensor(
                        out=t3, in0=h3,
                        in1=asr_s[:].unsqueeze(1).to_broadcast([P, d, DIM]),
                        op=ALU.mult)
                    sA = wpool.tile([P, d], F32, tag="sA")
                    nc.vector.tensor_reduce(
                        out=sA[:], in_=t3, axis=mybir.AxisListType.X, op=ALU.add)
                    # w = exp(leaky(as + ad))
                    nc.vector.tensor_tensor(
                        out=sA[:], in0=sA[:],
                        in1=ad_s[:, c:c + 1].to_broadcast([P, d]), op=ALU.add)
                    nc.vector.scalar_tensor_tensor(
                        out=sA[:], in0=sA[:], scalar=NEG_SLOPE, in1=sA[:],
                        op0=ALU.mult, op1=ALU.max)
                    nc.scalar.activation(out=sA[:], in_=sA[:], func=AF.Exp)
                    # den = sum(w * valid) (+ self term later)
                    wv = wpool.tile([P, d], F32, tag="wv")
                    nc.vector.tensor_tensor(
                        out=wv[:], in0=sA[:], in1=valid_s[:, A:A + d],
                        op=ALU.mult)
                    nc.vector.tensor_reduce(
                        out=den[:, j:j + 1], in_=wv[:],
                        axis=mybir.AxisListType.X, op=ALU.add)
                    # num = reduce_q(hsel * w)   (pads: hsel = 0)
                    wb = wpool.tile([P, d], BF16, tag="wb")
                    nc.vector.tensor_copy(out=wb[:], in_=sA[:])
                    nc.vector.tensor_tensor(
                        out=t3, in0=h3,
                        in1=wb[:].unsqueeze(2).to_broadcast([P, d, DIM]),
                        op=ALU.mult)
                    nc.vector.tensor_reduce(
                        out=opre[:, j * DIM:(j + 1) * DIM],
                        in_=tb[:].rearrange("p (q f) -> p f q", f=DIM),
                        axis=mybir.AxisListType.X, op=ALU.add)
                    # self-loop numerator term
                    nc.vector.scalar_tensor_tensor(
                        out=opre[:, j * DIM:(j + 1) * DIM],
                        in0=own3[:, j, 0:DIM], scalar=ws[:, j:j + 1],
                        in1=opre[:, j * DIM:(j + 1) * DIM],
                        op0=ALU.mult, op1=ALU.add)
                nc.vector.tensor_add(out=den[:], in0=den[:], in1=ws[:])
                nc.vector.tensor_scalar_add(out=den[:], in0=den[:], scalar1=EPS)
                nc.vector.reciprocal(out=rden[:], in_=den[:])
                z_g = npool.tile([P, nch * DIM], F32, tag="z")
                for j in range(nch):
                    nc.vector.scalar_tensor_tensor(
                        out=z_g[:, j * DIM:(j + 1) * DIM],
                        in0=opre[:, j * DIM:(j + 1) * DIM],
                        scalar=rden[:, j:j + 1], in1=b_s[:],
                        op0=ALU.mult, op1=ALU.add)
                # elu(x) = max(x, exp(min(x, 0)) - 1)
                mneg = npool.tile([P, nch * DIM], F32, tag="mneg")
                nc.vector.tensor_scalar_min(out=mneg[:], in0=z_g[:], scalar1=0.0)
                e = npool.tile([P, nch * DIM], F32, tag="e")
                nc.scalar.activation(out=e[:], in_=mneg[:], func=AF.Exp)
                nc.vector.scalar_tensor_tensor(
                    out=z_g[:], in0=e[:], scalar=-1.0, in1=z_g[:],
                    op0=ALU.add, op1=ALU.max)
                return z_g

            # ---- phase B ----
            for gi, (c0, c1) in enumerate(groups):
                nch = c1 - c0
                z_g = aggregate_group(gi, g1q, asr1_s, ad1_s, b1_s, own1_s)
                nc.sync.dma_start(
                    out=dram_rows(out_z, c0, nch, DIM),
                    in_=z_g[:].rearrange("p (c f) -> p c f", f=DIM))
                ps2 = psA.tile([P, nch * ROWF], F32, tag="psA")
                for h in range(0, nch, 2):
                    nh = min(2, nch - h)
                    ptT = psT.tile([nh * DIM, P], F32, tag="psT")
                    nc.tensor.transpose(
                        out=ptT[:], in_=z_g[:, h * DIM:(h + nh) * DIM],
                        identity=ident[:])
                    zT = stg.tile([nh * DIM, P], F32, tag="zT")
                    nc.vector.tensor_copy(out=zT[:], in_=ptT[:])
                    for j in range(nh):
                        nc.tensor.matmul(
                            out=ps2[:, (h + j) * ROWF:(h + j + 1) * ROWF],
                            lhsT=zT[j * DIM:(j + 1) * DIM, :],
                            rhs=m2_s[j * DIM:(j + 1) * DIM, :],
                            start=True, stop=True)
                gs2 = own2_s[:, c0 * ROWF:c1 * ROWF]
                nc.vector.tensor_copy(out=gs2, in_=ps2[:])
                ps23 = ps2[:].rearrange("p (c f) -> p c f", f=ROWF)
                nc.vector.tensor_copy(out=ad2_s[:, c0:c1], in_=ps23[:, :, COL_AD])
                hb2 = stg.tile([P, nch * DIM], BF16, tag="hb")
                nc.vector.tensor_copy(
                    out=hb2[:].rearrange("p (c f) -> p c f", f=DIM),
                    in_=ps23[:, :, 0:DIM])
                nc.sync.dma_start(
                    out=dram_rows(cc2, c0, nch, DIM),
                    in_=hb2[:].rearrange("p (c f) -> p c f", f=DIM))

            nc.gpsimd.collective_compute(
                "AllGather", ALU.bypass, replica_groups=rg,
                ins=[cc2], outs=[g2[0:ntot, :]])
            mirror(g2, g2p)

            # ---- phase C ----
            for gi, (c0, c1) in enumerate(groups):
                nch = c1 - c0
                x_g = aggregate_group(gi, g2q, asr2_s, ad2_s, b2_s, own2_s)
                nc.sync.dma_start(
                    out=dram_rows(out_x, c0, nch, DIM),
                    in_=x_g[:].rearrange("p (c f) -> p c f", f=DIM))

    nc.compile()
    return nc


# --------------------------------------------------------------------------
def make_in_maps(meta, x, W1, a_src1, a_dst1, b1, W2, a_src2, a_dst2, b2):
    rank, npc_raw, npc = meta["rank"], meta["npc_raw"], meta["NPC"]

    def maug(w, a_s, a_d):
        m = np.zeros((DIM, ROWF), dtype=np.float32)
        m[:, 0:DIM] = w
        m[:, COL_AS] = w @ a_s
        m[:, COL_AD] = w @ a_d
        return m

    m1 = maug(W1, a_src1, a_dst1)
    m2 = np.tile(maug(W2, a_src2, a_dst2), (2, 1))
    asr1 = np.tile(a_src1.astype(np.float32)[None, :], (P, 1)).astype(
        ml_dtypes.bfloat16)
    asr2 = np.tile(a_src2.astype(np.float32)[None, :], (P, 1)).astype(
        ml_dtypes.bfloat16)
    b1r = np.tile(b1.astype(np.float32)[None, :], (P, 1))
    b2r = np.tile(b2.astype(np.float32)[None, :], (P, 1))

    xm = x[rank]
    in_maps = []
    for c in range(C):
        xc = xm[c::C]
        xT = np.zeros((DIM, npc), dtype=np.float32)
        xT[:, :npc_raw] = xc.T
        in_maps.append({
            "xT": np.ascontiguousarray(xT),
            "qidx": np.ascontiguousarray(meta["qidx"][c]),
            "masks": np.ascontiguousarray(meta["masks"][c]),
            "valid": np.ascontiguousarray(meta["valid"][c]),
            "m1aug": m1, "m2aug": m2, "asr1": asr1, "asr2": asr2,
            "b1rep": b1r, "b2rep": b2r,
        })
    return in_maps


def unshard(meta, results, n_nodes):
    rank, npc_raw = meta["rank"], meta["npc_raw"]
    z = np.empty((n_nodes, DIM), dtype=np.float32)
    xbar = np.empty((n_nodes, DIM), dtype=np.float32)
    for c in range(C):
        ids = rank[np.arange(npc_raw) * C + c]
        z[ids] = results[c]["out_z"][:npc_raw]
        xbar[ids] = results[c]["out_xbar"][:npc_raw]
    return xbar, z


_CACHE: dict = {}


def kernel(x, edge_index, W1, a_src1, a_dst1, b1, W2, a_src2, a_dst2, b2):
    x = np.asarray(x, dtype=np.float32)
    edge_index = np.asarray(edge_index)
    args = [np.asarray(a, dtype=np.float32)
            for a in (W1, a_src1, a_dst1, b1, W2, a_src2, a_dst2, b2)]
    n_nodes = x.shape[0]

    import time as _time
    t0 = _time.time()
    meta = partition_graph(edge_index, n_nodes)
    in_maps = make_in_maps(meta, x, *args)
    t1 = _time.time()

    key = (n_nodes, meta["S"], tuple(meta["Dk"].tolist()))
    nc = _CACHE.get(key)
    if nc is None:
        nc = build_program(meta)
        _CACHE[key] = nc
    t2 = _time.time()
    print(f"[kernel] partition {t1 - t0:.1f}s  build+compile {t2 - t1:.1f}s",
          flush=True)

    res = run_bass_kernel_spmd(nc, in_maps, list(range(C)), **RUN_KWARGS)
    xbar, z = unshard(meta, res.results, n_nodes)
    if RUN_KWARGS:
        kernel.last_result = res  # type: ignore[attr-defined]
    return xbar, z
